# revision 21
# baseline (speedup 1.0000x reference)
"""Trainium2 Bass kernel for nn_ConvZero GNN message passing (8 NeuronCores).

Strategy (edge/data parallel, per sharding hint):
- Host shards edges by destination-node bucket (12500 nodes/core), sorts each
  shard by dst, pads each 128-node window's edge run so all 8 cores share ONE
  static edge-tile -> node-window schedule (SPMD). Host stages transposed bf16
  streams: gathered src features, edge features, dst-selector one-hots (both
  orientations) packed per chunk (one DMA issue per tensor per chunk).
- Device pass 1 (single compute of messages): per 128-edge tile, 4 PSUM-
  accumulated matmuls m[e,f] = x_src@W1 + oh@B_win + attr_aug@We_aug + erep@W3
  (B_win = node_window@W2 computed per window on the fly), 4 tiles per PSUM
  group, one batched Act copy per group into a resident SBUF slab (bf16,
  129-column tile stride with interleaved ones columns). BN stats are ONE
  Gram-matmul per tile (rhs = [m_tile | ones] -> ssq = diag, ssum = last col)
  accumulated in PSUM -> AllReduce [128,2] -> c = bn_b/gam - mu, gam folded
  into Wm1.
- Device pass 2: rm = relu(m + c) on DVE (two batched ops per 4-tile group),
  scatter-add y^T[f,n] via host-staged [e,n] one-hot matmuls, PSUM-accumulated
  per window; y^T spilled to DRAM (frees SBUF for the MLP).
- MLP in transposed layout [feat, node]: z1/z2/h1 slabs reuse the message
  slab's SBUF, BN stats via Act accum_out riding the PSUM->SBUF copies plus
  Pool squares + DVE reductions, AllReduce'd per layer.
- DMA issues round-robin across the sync/scalar/gpsimd hardware queues.
"""
import sys
sys.path.insert(0, "/opt/trn_rl_repo")
import numpy as np
import ml_dtypes

import concourse.bass as bass
from concourse import bacc
import concourse.mybir as mybir
from concourse.tile import TileContext
from concourse import bass_utils
from concourse.masks import make_identity

BF16 = ml_dtypes.bfloat16
FP8 = ml_dtypes.float8_e4m3
F32 = np.float32
DT = mybir.dt.bfloat16
F8 = mybir.dt.float8e4
FP = mybir.dt.float32

N, E, H, ED = 100000, 640000, 128, 16
EPS = 1e-5
NCORES = 8
NB = N // NCORES            # 12500
NBT = (NB + 127) // 128     # 98 node windows per core
NBP = NBT * 128             # 12544
CH = 4                      # tiles per stream chunk / psum group
MLP_NBLK = [(i * 512, min(NBP, (i + 1) * 512)) for i in range((NBP + 511) // 512)]

_CACHE = {}


def _host_prep(inputs):
    src = np.asarray(inputs["edge_index"][0]).astype(np.int64)
    dst = np.asarray(inputs["edge_index"][1]).astype(np.int64)
    node_rep = np.asarray(inputs["node_rep"], dtype=F32)
    edge_rep = np.asarray(inputs["edge_rep"], dtype=F32)
    edge_attr = np.asarray(inputs["edge_attr"], dtype=F32)

    core_of = np.minimum(dst // NB, NCORES - 1)
    percore = []
    counts = np.zeros((NCORES, NBT), dtype=np.int64)
    for c in range(NCORES):
        eids = np.nonzero(core_of == c)[0]
        dl = dst[eids] - c * NB
        order = np.argsort(dl, kind="stable")
        eids = eids[order]
        dl = dl[order]
        counts[c] = np.bincount(dl // 128, minlength=NBT)
        percore.append((eids, dl))
    T_k = np.maximum(np.ceil(counts.max(axis=0) / 128).astype(np.int64), 1)
    NT = int(T_k.sum())
    extra = (-NT) % CH      # pad tile count to a chunk multiple
    T_k[NBT - 1] += extra
    NT += extra
    EP = NT * 128
    NCH = NT // CH
    sched = np.repeat(np.arange(NBT), T_k)
    tile_start = (np.concatenate([[0], np.cumsum(T_k)[:-1]]) * 128)

    cores = []
    for c in range(NCORES):
        eids, dl = percore[c]
        pos = np.zeros(len(eids), dtype=np.int64)
        start = 0
        for k in range(NBT):
            n_k = counts[c, k]
            pos[start:start + n_k] = tile_start[k] + np.arange(n_k)
            start += n_k
        # per-edge-slot streams in [feat, edge] layout
        x_srcT = np.zeros((H, EP), dtype=BF16)
        x_srcT[:, pos] = node_rep[src[eids]].T
        erepT = np.zeros((H, EP), dtype=BF16)
        erepT[:, pos] = edge_rep[eids].T
        attrT = np.zeros((ED + 1, EP), dtype=BF16)
        attrT[:ED, pos] = edge_attr[eids].T
        attrT[ED, pos] = 1.0
        dl_pad = np.full(EP, -1, dtype=np.int64)
        dl_pad[pos] = dl
        tilenos = np.arange(EP) // 128
        nl = dl_pad - sched[tilenos] * 128      # local idx in window, -1 pad
        ok = (nl >= 0) & (nl < 128)
        oh_ne = np.zeros((128, EP), dtype=FP8)
        oh_ne[nl[ok], np.arange(EP)[ok]] = 1.0
        e_in_tile = np.arange(EP) % 128
        oh_en = np.zeros((128, EP), dtype=FP8)
        oh_en[e_in_tile[ok], tilenos[ok] * 128 + nl[ok]] = 1.0
        # pack xs|erep per chunk: big2[r, ch, 2*CH*128]; one-hots fp8
        W = CH * 128
        NCH_l = NT // CH
        big2 = np.empty((128, NCH_l, 2 * W), dtype=BF16)
        big2[:, :, 0 * W:1 * W] = x_srcT.reshape(H, NCH_l, W)
        big2[:, :, 1 * W:2 * W] = erepT.reshape(H, NCH_l, W)
        ohne = oh_ne.reshape(128, NCH_l, W).copy()
        small = attrT.reshape(ED + 1, NCH_l, W).copy()
        ohen = oh_en.reshape(128, NCH_l, W).copy()
        nbT = np.zeros((H, NBP), dtype=BF16)
        hi = min((c + 1) * NB, N) - c * NB
        nbT[:, :hi] = node_rep[c * NB:c * NB + hi].T
        cores.append(dict(big2=big2, ohne=ohne, small=small, ohen=ohen,
                          nbT=nbT))
    return cores, sched, NT, EP


def _build(NT, sched):
    NCH = NT // CH
    W = CH * 128
    TS = 129                      # m_slab per-tile column stride
    nc = bacc.Bacc("TRN2", target_bir_lowering=False, debug=False,
                   num_devices=NCORES)
    DI = lambda name, shape, dt=DT: nc.dram_tensor(name, shape, dt,
                                                   kind="ExternalInput")
    big2 = DI("big2", [128, NCH, 2 * W])
    ohne_d = DI("ohne", [128, NCH, W], F8)
    small = DI("small", [ED + 1, NCH, W])
    ohen_d = DI("ohen", [128, NCH, W], F8)
    nbT = DI("nbT", [H, NBP])
    W1 = DI("W1", [H, H])
    W2 = DI("W2", [H, H])
    W3 = DI("W3", [H, H])
    We_aug = DI("We_aug", [ED + 1, H])
    Wm1 = DI("Wm1", [H, 2 * H])
    Wm2p = DI("Wm2p", [H, 2 * 2 * H])
    Wm3p = DI("Wm3p", [H, 2 * H])
    vecs = DI("vecs", [128, 8], FP)   # col0 bn_g, col1 bn_b, col2-3 g1 halves,
    # col4-5 b1 halves, col6 bm3
    vecs2 = DI("vecs2", [128, 4], FP)  # g2 halves, b2 halves
    yout = nc.dram_tensor("yout", [128, NBP], DT, kind="ExternalOutput")

    # window segments in the tile schedule: (window, t0, t1)
    segs = []
    t = 0
    while t < NT:
        t1 = t
        while t1 < NT and sched[t1] == sched[t]:
            t1 += 1
        segs.append((int(sched[t]), t, t1))
        t = t1
    win_first = {ta: i for i, (k, ta, tb) in enumerate(segs)}

    with TileContext(nc) as tc:
        with (
            tc.tile_pool(name="const", bufs=1) as constp,
            tc.tile_pool(name="slab", bufs=1) as slabp,
            tc.tile_pool(name="stream", bufs=3) as streamp,
            tc.tile_pool(name="bigstr", bufs=4) as bigstrp,
            tc.tile_pool(name="ohstr", bufs=6) as ohstrp,
            tc.tile_pool(name="work", bufs=2) as workp,
            tc.tile_pool(name="mps", bufs=2, space="PSUM") as mpsp,
            tc.tile_pool(name="acc", bufs=1, space="PSUM") as accp,
            tc.tile_pool(name="yps", bufs=2, space="PSUM") as ypsp,
            tc.tile_pool(name="zps", bufs=2, space="PSUM") as zpsp,
            tc.tile_pool(name="dram", bufs=1, space="DRAM") as dramp,
        ):
            f32 = FP
            dmae = [nc.sync, nc.scalar, nc.gpsimd]

            # ---- constants ----
            W1s = constp.tile([H, H], DT); nc.sync.dma_start(W1s[:], W1[:, :])
            W2s = constp.tile([H, H], DT); nc.sync.dma_start(W2s[:], W2[:, :])
            W3s = constp.tile([H, H], DT); nc.sync.dma_start(W3s[:], W3[:, :])
            Wes = constp.tile([128, H], DT)
            nc.vector.memset(Wes[:], 0.0)
            nc.sync.dma_start(Wes[0:ED + 1, :], We_aug[:, :])
            Wm1s = constp.tile([H, 2 * H], DT)
            nc.scalar.dma_start(Wm1s[:], Wm1[:, :])
            Wm2s = constp.tile([H, 4 * H], DT)
            nc.scalar.dma_start(Wm2s[:], Wm2p[:, :])
            Wm3s = constp.tile([H, 2 * H], DT)
            nc.scalar.dma_start(Wm3s[:], Wm3p[:, :])
            vec = constp.tile([128, 8], f32)
            nc.gpsimd.dma_start(vec[:], vecs[:, :])
            vec2 = constp.tile([128, 4], f32)
            nc.gpsimd.dma_start(vec2[:], vecs2[:, :])
            ident = constp.tile([128, 128], f32)
            make_identity(nc, ident[:])
            wu_sb = constp.tile([128, 2], f32, tag="wu")
            nc.vector.memset(wu_sb[:], 0.0)
            wu_in = dramp.tile([128, 2], f32, tag="wui")
            wu_out = dramp.tile([128, 2], f32, tag="wuo")
            nc.scalar.dma_start(wu_in[:], wu_sb[:])
            nc.gpsimd.collective_compute(
                "AllReduce", mybir.AluOpType.add,
                ins=[wu_in.opt()], outs=[wu_out.opt()],
                replica_groups=[list(range(NCORES))])
            ones_col = constp.tile([128, 1], DT)
            nc.vector.memset(ones_col[:], 1.0)

            # message slab, tile stride 129: [m(128) | ones(1)] per tile.
            # Reused by the MLP z1/z2/h1 slabs afterwards.
            m_slab = slabp.tile([128, NT * TS], DT)
            nc.vector.memset(
                m_slab[:].rearrange("p (t c) -> p t c", c=TS)[:, :, 128:129],
                1.0)

            def m_t(t):
                return m_slab[:, t * TS: t * TS + 128]

            # ---- pass 1: single m computation + Gram stats ----
            gram_a = accp.tile([128, TS], f32, tag="grama")
            gram_b = accp.tile([128, TS], f32, tag="gramb")
            HALF = (NT // 2 // CH) * CH

            def stats_mm(t):
                gp = gram_a if t < HALF else gram_b
                lo, hi = (0, HALF - 1) if t < HALF else (HALF, NT - 1)
                nc.tensor.matmul(gp[:], lhsT=m_t(t),
                                 rhs=m_slab[:, t * TS: t * TS + TS],
                                 start=(t == lo), stop=(t == hi))

            def compute_B(w, nbtile):
                bp = ypsp.tile([128, 128], f32, tag="yps", name="bps")
                nc.tensor.matmul(bp[:], lhsT=nbtile[:], rhs=W2s[:],
                                 start=True, stop=True)
                bw = workp.tile([128, 128], DT, tag="bwin", name="bwin")
                nc.scalar.copy(bw[:], bp[:])
                return bw

            small_bufs = []
            for _ in range(4):
                stt = bigstrp.tile([128, W], DT, tag="small")
                nc.vector.memset(stt[:], 0.0)
                small_bufs.append(stt)

            def issue_chunk(ch):
                bt = bigstrp.tile([128, 2 * W], DT, tag="big2")
                dmae[ch % 3].dma_start(bt[:], big2[:, ch, :])
                ot = bigstrp.tile([128, W], F8, tag="ohne")
                dmae[(ch + 1) % 3].dma_start(ot[:], ohne_d[:, ch, :])
                st = small_bufs[ch % 4]
                dmae[(ch + 2) % 3].dma_start(st[0:ED + 1, :],
                                             small[:, ch, :])
                return bt, ot, st

            def issue_nb(wi):
                k = segs[wi][0]
                nbt = streamp.tile([H, 128], DT, tag="nb")
                dmae[wi % 3].dma_start(nbt[:], nbT[:, k * 128:(k + 1) * 128])
                return nbt

            dscr_a = constp.tile([128, 128], f32, tag="dscra")
            sta_sb = constp.tile([128, 2], f32, tag="sta")
            cca_in = dramp.tile([128, 2], f32, tag="ccai")
            cca_out = dramp.tile([128, 2], f32, tag="ccao")
            chunk_t = {c: issue_chunk(c) for c in range(min(3, NCH))}
            nb_tl = {w: issue_nb(w) for w in range(min(2, len(segs)))}
            Bw = {0: compute_B(0, nb_tl[0])}
            Bwin = None

            for g in range(NCH):
                if g + 3 < NCH:
                    chunk_t[g + 3] = issue_chunk(g + 3)
                    chunk_t.pop(g - 1, None)
                big2_t, ohne_t, small_t = chunk_t[g]
                mpg = mpsp.tile([128, W], f32, tag="mps")
                for j in range(CH):
                    t = g * CH + j
                    if t in win_first:
                        wi = win_first[t]
                        if wi + 2 < len(segs):
                            nb_tl[wi + 2] = issue_nb(wi + 2)
                            nb_tl.pop(wi - 1, None)
                        if wi + 1 < len(segs):
                            Bw[wi + 1] = compute_B(wi + 1, nb_tl[wi + 1])
                        Bwin = Bw.pop(wi)
                    xs_sl = big2_t[:, 0 * W + j * 128: 0 * W + (j + 1) * 128]
                    es_sl = big2_t[:, 1 * W + j * 128: 1 * W + (j + 1) * 128]
                    oh_sl = ohne_t[:, j * 128:(j + 1) * 128]
                    at_sl = small_t[:, j * 128:(j + 1) * 128]
                    mp = mpg[:, j * 128:(j + 1) * 128]
                    nc.tensor.matmul(mp, lhsT=xs_sl, rhs=W1s[:],
                                     start=True, stop=False)
                    nc.tensor.matmul(mp, lhsT=es_sl, rhs=W3s[:],
                                     start=False, stop=False)
                    nc.tensor.matmul(mp, lhsT=at_sl, rhs=Wes[:],
                                     start=False, stop=False)
                    nc.tensor.matmul(mp, lhsT=oh_sl, rhs=Bwin[:],
                                     start=False, stop=True)
                # one batched copy into the strided slab
                nc.scalar.copy(
                    m_slab[:, g * CH * TS:(g + 1) * CH * TS]
                        .rearrange("p (t c) -> p t c", c=TS)[:, :, 0:128],
                    mpg[:].rearrange("p (t c) -> p t c", c=128))
                if g >= 1:
                    for j in range(CH):
                        stats_mm((g - 1) * CH + j)
                    if g * CH == HALF:
                        nc.vector.tensor_mul(dscr_a[:], gram_a[:, 0:128],
                                             ident[:])
                        nc.vector.reduce_sum(sta_sb[:, 1:2], dscr_a[:],
                                             axis=mybir.AxisListType.X)
                        nc.vector.tensor_copy(sta_sb[:, 0:1],
                                              gram_a[:, 128:129])
                        nc.sync.dma_start(cca_in[:], sta_sb[:])
                        nc.gpsimd.collective_compute(
                            "AllReduce", mybir.AluOpType.add,
                            ins=[cca_in.opt()], outs=[cca_out.opt()],
                            replica_groups=[list(range(NCORES))])
            for j in range(CH):
                stats_mm((NCH - 1) * CH + j)

            # ---- stats: half-B AR, combine with (already inflight) half-A ----
            st_sb = constp.tile([128, 2], f32, tag="st")
            dscr = constp.tile([128, 128], f32, tag="dscr")
            nc.vector.tensor_mul(dscr[:], gram_b[:, 0:128], ident[:])
            nc.vector.reduce_sum(st_sb[:, 1:2], dscr[:],
                                 axis=mybir.AxisListType.X)
            nc.vector.tensor_copy(st_sb[:, 0:1], gram_b[:, 128:129])
            cc_in = dramp.tile([128, 2], f32, tag="cci")
            cc_out = dramp.tile([128, 2], f32, tag="cco")
            nc.sync.dma_start(cc_in[:], st_sb[:])
            nc.gpsimd.collective_compute(
                "AllReduce", mybir.AluOpType.add,
                ins=[cc_in.opt()], outs=[cc_out.opt()],
                replica_groups=[list(range(NCORES))])
            stg = constp.tile([128, 2], f32, tag="stg")
            nc.sync.dma_start(stg[:], cc_out[:])
            stga = constp.tile([128, 2], f32, tag="stga")
            nc.scalar.dma_start(stga[:], cca_out[:])
            nc.vector.tensor_add(stg[:], stg[:], stga[:])

            # mu = S1/E ; var = S2/E - mu^2 ; gam = bn_g*rstd
            # c = bn_b/gam - mu (requires bn_g > 0, true here)
            tmp = constp.tile([128, 6], f32, tag="bn")
            mu = tmp[:, 0:1]; var = tmp[:, 1:2]; gam = tmp[:, 2:3]
            cvec = tmp[:, 3:4]; r = tmp[:, 4:5]; t5 = tmp[:, 5:6]
            nc.vector.tensor_scalar_mul(mu, stg[:, 0:1], 1.0 / E)
            nc.vector.tensor_scalar_mul(var, stg[:, 1:2], 1.0 / E)
            nc.scalar.square(t5, mu)
            nc.vector.tensor_sub(var, var, t5)
            nc.vector.tensor_scalar_add(var, var, EPS)
            nc.vector.reciprocal(r, var)
            nc.scalar.sqrt(r, r)                       # r = rstd
            nc.vector.tensor_mul(gam, vec[:, 0:1], r)  # gam = g * rstd
            nc.vector.reciprocal(t5, gam)
            nc.vector.tensor_mul(t5, vec[:, 1:2], t5)  # b / gam
            nc.vector.tensor_sub(cvec, t5, mu)         # c = b/gam - mu
            # broadcast c across partitions: c_bc[e, f] = c[f]
            cb_ps = ypsp.tile([128, 128], f32, tag="yps", name="cb_ps")
            nc.tensor.transpose(cb_ps[:], cvec.to_broadcast([128, 128]),
                                ident[:])
            c4 = constp.tile([128, W], DT, tag="c4")
            for j in range(CH):
                nc.scalar.copy(c4[:, j * 128:(j + 1) * 128], cb_ps[:])
            # fold gam into Wm1 rows: Wm1g[f, :] = gam[f] * Wm1[f, :]
            Wm1g = constp.tile([H, 2 * H], DT, tag="wm1g")
            nc.vector.tensor_scalar_mul(Wm1g[:], Wm1s[:], gam)

            # MLP layer-1 slabs/stats (interleaved into pass 2)
            z1_sb = [m_slab[:, 0:NBP], m_slab[:, NBP:2 * NBP]]
            nblk = len(MLP_NBLK)
            cols1 = constp.tile([128, 4 * nblk], f32, tag="colsz1",
                                name="colsz1")
            sqacc1 = [constp.tile([128, 512], DT, tag=f"sqa1_{hh}",
                                  name=f"sqa1_{hh}") for hh in range(2)]
            for hh in range(2):
                nc.vector.memset(sqacc1[hh][:], 0.0)

            # ---- pass 2: rm = relu(m + c), scatter to yT[f, n] ----
            yT_dram = dramp.tile([128, NBP], DT, tag="ytd")
            NG = NCH

            def issue_ohen(g):
                ot = ohstrp.tile([128, W], F8, tag="ohen")
                dmae[g % 3].dma_start(ot[:], ohen_d[:, g, :])
                return ot

            def gen_rm(g):
                rm = workp.tile([128, W], DT, tag="rm", name="rm")
                eng = nc.gpsimd if g % 3 == 2 else nc.vector
                eng.tensor_add(
                    rm[:].rearrange("p (t c) -> p t c", c=128),
                    m_slab[:, g * CH * TS:(g + 1) * CH * TS]
                        .rearrange("p (t c) -> p t c", c=TS)[:, :, 0:128],
                    c4[:].rearrange("p (t c) -> p t c", c=128))
                nc.vector.tensor_scalar_max(rm[:], rm[:], 0.0)
                return rm

            ohen_t = {g: issue_ohen(g) for g in range(min(6, NG))}
            rm_g = {0: gen_rm(0)}
            # first segment index after which all m tiles that the z1 slabs
            # overwrite (tiles < z1_hi) have been consumed by the scatter
            z1_hi = (2 * NBP + TS - 1) // TS + 1
            si_start = next(i for i, (_, _, tb) in enumerate(segs)
                            if tb >= z1_hi)

            def z1_block(i):
                a, b = MLP_NBLK[i]
                yk = workp.tile([128, 512], DT, tag="ybk", name="ybk")
                dmae[i % 3].dma_start(yk[:, :b - a], yT_dram[:, a:b])
                for hh in range(2):
                    zp = zpsp.tile([128, 512], f32, tag="zps", name="z1ps")
                    nc.tensor.matmul(zp[:, :b - a],
                                     lhsT=Wm1g[:, hh * 128:(hh + 1) * 128],
                                     rhs=yk[:, :b - a],
                                     start=True, stop=True)
                    nc.scalar.activation(
                        z1_sb[hh][:, a:b], zp[:, :b - a],
                        mybir.ActivationFunctionType.Identity,
                        accum_out=cols1[:, 4 * i + 2 * hh:
                                        4 * i + 2 * hh + 1])
                    scr = workp.tile([128, W], DT, tag="rm", name="scr")
                    nc.vector.tensor_mul(scr[:, :b - a], z1_sb[hh][:, a:b],
                                         z1_sb[hh][:, a:b])
                    nc.vector.tensor_add(sqacc1[hh][:, :b - a],
                                         sqacc1[hh][:, :b - a],
                                         scr[:, :b - a])

            for si, (k, ta, tb) in enumerate(segs):
                yp = ypsp.tile([128, 128], f32, tag="yps", name="yps")
                for t in range(ta, tb):
                    g, j = divmod(t, CH)
                    if j == 0 and g + 1 < NG and (g + 1) not in rm_g:
                        rm_g[g + 1] = gen_rm(g + 1)
                        rm_g.pop(g - 1, None)
                        if g + 6 < NG:
                            ohen_t[g + 6] = issue_ohen(g + 6)
                            ohen_t.pop(g - 1, None)
                    rm = rm_g[g]
                    nc.tensor.matmul(yp[:], lhsT=rm[:, j * 128:(j + 1) * 128],
                                     rhs=ohen_t[g][:, j * 128:(j + 1) * 128],
                                     start=(t == ta), stop=(t == tb - 1))
                yb = workp.tile([128, 128], DT, tag="yb", name="yb")
                nc.scalar.copy(yb[:], yp[:])
                dmae[si % 3].dma_start(yT_dram[:, k * 128:(k + 1) * 128],
                                       yb[:])
                if si >= si_start and (si - si_start) % 3 == 0 \
                        and (si - si_start) // 3 < nblk:
                    z1_block((si - si_start) // 3)

            # ---- MLP (transposed layout [feat, node]) ----
            z2_sb = [m_slab[:, 2 * NBP:3 * NBP], m_slab[:, 3 * NBP:4 * NBP]]
            h1_sb = [m_slab[:, 4 * NBP:5 * NBP], m_slab[:, 5 * NBP:6 * NBP]]

            def bn_coeffs(stz, gcols, bcols, tag):
                out = constp.tile([128, 4], f32, tag=f"bncf{tag}")
                wrk = constp.tile([128, 2], f32, tag=f"bnw{tag}")
                for hh in range(2):
                    muz = wrk[:, 0:1]; vz = wrk[:, 1:2]
                    ga = out[:, 2 * hh:2 * hh + 1]
                    be_ = out[:, 2 * hh + 1:2 * hh + 2]
                    nc.vector.tensor_scalar_mul(muz, stz[:, 2 * hh:2 * hh + 1],
                                                1.0 / N)
                    nc.vector.tensor_scalar_mul(
                        vz, stz[:, 2 * hh + 1:2 * hh + 2], 1.0 / N)
                    nc.scalar.square(ga, muz)
                    nc.vector.tensor_sub(vz, vz, ga)
                    nc.vector.tensor_scalar_add(vz, vz, EPS)
                    nc.vector.reciprocal(vz, vz)
                    nc.scalar.sqrt(vz, vz)
                    nc.vector.tensor_mul(ga, gcols[hh], vz)
                    nc.vector.tensor_mul(be_, ga, muz)
                    nc.vector.tensor_sub(be_, bcols[hh], be_)
                return out

            # --- layer 1 leftovers (most blocks ran inside pass 2) ---
            done1 = max(0, min(nblk,
                               (len(segs) - 1 - si_start) // 3 + 1))
            for i in range(done1, nblk):
                z1_block(i)
            acc1 = constp.tile([128, 4], f32, tag="accz1")
            for hh in range(2):
                nc.vector.reduce_sum(
                    acc1[:, 2 * hh:2 * hh + 1],
                    cols1[:].rearrange("p (i j) -> p i j", j=4)[:, :, 2 * hh],
                    axis=mybir.AxisListType.X)
                nc.vector.reduce_sum(acc1[:, 2 * hh + 1:2 * hh + 2],
                                     sqacc1[hh][:],
                                     axis=mybir.AxisListType.X)
            ci1 = dramp.tile([128, 4], f32, tag="ciz1")
            co1 = dramp.tile([128, 4], f32, tag="coz1")
            nc.sync.dma_start(ci1[:], acc1[:])
            nc.gpsimd.collective_compute(
                "AllReduce", mybir.AluOpType.add,
                ins=[ci1.opt()], outs=[co1.opt()],
                replica_groups=[list(range(NCORES))])
            stz1 = constp.tile([128, 4], f32, tag="stzz1")
            nc.sync.dma_start(stz1[:], co1[:])
            cf1 = bn_coeffs(stz1, [vec[:, 2:3], vec[:, 3:4]],
                            [vec[:, 4:5], vec[:, 5:6]], "z1")

            # h1 = ga*max(z1, Q) - ga*Q with Q = -be/ga; the -ga*Q = be term
            # shifts z2 by a constant vector, which BN2 cancels. ga folds into
            # Wm2 rows. Apply max in place on z1 (one DVE op per block).
            q1 = constp.tile([128, 4], f32, tag="q1")
            Wm2g = constp.tile([H, 4 * H], DT, tag="wm2g")
            for hh in range(2):
                nc.vector.reciprocal(q1[:, hh + 2:hh + 3],
                                     cf1[:, 2 * hh:2 * hh + 1])
                nc.vector.tensor_mul(q1[:, hh:hh + 1],
                                     cf1[:, 2 * hh + 1:2 * hh + 2],
                                     q1[:, hh + 2:hh + 3])
                nc.vector.tensor_scalar_mul(q1[:, hh:hh + 1],
                                            q1[:, hh:hh + 1], -1.0)
                nc.vector.tensor_scalar_mul(
                    Wm2g[:, hh * 256:(hh + 1) * 256],
                    Wm2s[:, hh * 256:(hh + 1) * 256],
                    cf1[:, 2 * hh:2 * hh + 1])
                for (a, b) in MLP_NBLK:
                    nc.vector.tensor_scalar_max(z1_sb[hh][:, a:b],
                                                z1_sb[hh][:, a:b],
                                                q1[:, hh:hh + 1])
                nc.vector.memset(z1_sb[hh][:, NB:NBP], 0.0)
            h1_sb = z1_sb

            # --- layer 2: z2 = Wm2^T @ h1 ---
            cols2 = constp.tile([128, 4 * nblk], f32, tag="colsz2",
                                name="colsz2")
            sqacc2 = sqacc1
            for gg in range(2):
                nc.vector.memset(sqacc2[gg][:], 0.0)
            for i, (a, b) in enumerate(MLP_NBLK):
                for gg in range(2):
                    zp = zpsp.tile([128, 512], f32, tag="zps", name="z2ps")
                    for hh in range(2):
                        nc.tensor.matmul(
                            zp[:, :b - a],
                            lhsT=Wm2g[:, hh * 256 + gg * 128:
                                      hh * 256 + gg * 128 + 128],
                            rhs=h1_sb[hh][:, a:b],
                            start=(hh == 0), stop=(hh == 1))
                    nc.scalar.activation(
                        z2_sb[gg][:, a:b], zp[:, :b - a],
                        mybir.ActivationFunctionType.Identity,
                        accum_out=cols2[:, 4 * i + 2 * gg:
                                        4 * i + 2 * gg + 1])
                    scr = workp.tile([128, W], DT, tag="rm", name="scr2")
                    nc.vector.tensor_mul(scr[:, :b - a], z2_sb[gg][:, a:b],
                                         z2_sb[gg][:, a:b])
                    nc.vector.tensor_add(sqacc2[gg][:, :b - a],
                                         sqacc2[gg][:, :b - a],
                                         scr[:, :b - a])
            acc2 = constp.tile([128, 4], f32, tag="accz2")
            for gg in range(2):
                nc.vector.reduce_sum(
                    acc2[:, 2 * gg:2 * gg + 1],
                    cols2[:].rearrange("p (i j) -> p i j", j=4)[:, :, 2 * gg],
                    axis=mybir.AxisListType.X)
                nc.vector.reduce_sum(acc2[:, 2 * gg + 1:2 * gg + 2],
                                     sqacc2[gg][:],
                                     axis=mybir.AxisListType.X)
            ci2 = dramp.tile([128, 4], f32, tag="ciz2")
            co2 = dramp.tile([128, 4], f32, tag="coz2")
            nc.sync.dma_start(ci2[:], acc2[:])
            nc.gpsimd.collective_compute(
                "AllReduce", mybir.AluOpType.add,
                ins=[ci2.opt()], outs=[co2.opt()],
                replica_groups=[list(range(NCORES))])
            stz2 = constp.tile([128, 4], f32, tag="stzz2")
            nc.sync.dma_start(stz2[:], co2[:])
            cf2 = bn_coeffs(stz2, [vec2[:, 0:1], vec2[:, 1:2]],
                            [vec2[:, 2:3], vec2[:, 3:4]], "z2")

            # --- layer 2 apply (in-place max) + layer 3 + bias -> out ---
            q2 = constp.tile([128, 4], f32, tag="q2")
            Wm3g = constp.tile([H, 2 * H], DT, tag="wm3g")
            be2b = constp.tile([128, 2], DT, tag="nbe2")
            for gg in range(2):
                nc.vector.reciprocal(q2[:, gg + 2:gg + 3],
                                     cf2[:, 2 * gg:2 * gg + 1])
                nc.vector.tensor_mul(q2[:, gg:gg + 1],
                                     cf2[:, 2 * gg + 1:2 * gg + 2],
                                     q2[:, gg + 2:gg + 3])
                nc.vector.tensor_scalar_mul(q2[:, gg:gg + 1],
                                            q2[:, gg:gg + 1], -1.0)
                nc.vector.tensor_scalar_mul(
                    Wm3g[:, gg * 128:(gg + 1) * 128],
                    Wm3s[:, gg * 128:(gg + 1) * 128],
                    cf2[:, 2 * gg:2 * gg + 1])
                nc.vector.tensor_copy(be2b[:, gg:gg + 1],
                                      cf2[:, 2 * gg + 1:2 * gg + 2])
                for (a, b) in MLP_NBLK:
                    nc.vector.tensor_scalar_max(z2_sb[gg][:, a:b],
                                                z2_sb[gg][:, a:b],
                                                q2[:, gg:gg + 1])
                nc.vector.memset(z2_sb[gg][:, NB:NBP], 0.0)
            # bias_col = bm3 + sum_g2 Wm3[g2,:] * be2[g2]
            vb_ps = ypsp.tile([128, 128], f32, tag="yps", name="vb")
            for gg in range(2):
                nc.tensor.matmul(vb_ps[:, 0:1],
                                 lhsT=Wm3s[:, gg * 128:(gg + 1) * 128],
                                 rhs=be2b[:, gg:gg + 1],
                                 start=(gg == 0), stop=(gg == 1))
            bias_col = constp.tile([128, 1], f32, tag="bcol")
            nc.vector.tensor_add(bias_col[:], vec[:, 6:7], vb_ps[:, 0:1])
            for i, (a, b) in enumerate(MLP_NBLK):
                ops = zpsp.tile([128, 512], f32, tag="zps", name="z3ps")
                for gg in range(2):
                    nc.tensor.matmul(ops[:, :b - a],
                                     lhsT=Wm3g[:, gg * 128:(gg + 1) * 128],
                                     rhs=z2_sb[gg][:, a:b],
                                     start=(gg == 0), stop=(gg == 1))
                ob = workp.tile([128, 512], DT, tag="ybk", name="ob")
                nc.scalar.activation(ob[:, :b - a], ops[:, :b - a],
                                     mybir.ActivationFunctionType.Identity,
                                     bias=bias_col[:])
                dmae[i % 3].dma_start(yout[:, a:b], ob[:, :b - a])

    nc.compile()
    return nc


def kernel(**inputs) -> np.ndarray:
    cores, sched, NT, EP = _host_prep(inputs)
    key = (NT, EP, tuple(sched[::37]))
    if key in _CACHE:
        nc = _CACHE[key]
    else:
        nc = _build(NT, sched)
        _CACHE[key] = nc

    bf = lambda x: np.asarray(x).astype(BF16)
    We = np.asarray(inputs["We"], dtype=F32)
    be = np.asarray(inputs["be"], dtype=F32)
    We_aug = np.concatenate([We, be[None, :]], axis=0).astype(BF16)
    Wm2 = np.asarray(inputs["Wm2"], dtype=F32)
    Wm2p = np.concatenate([Wm2[:128, :], Wm2[128:, :]], axis=1).astype(BF16)
    Wm3 = np.asarray(inputs["Wm3"], dtype=F32)
    Wm3p = np.concatenate([Wm3[:128, :], Wm3[128:, :]], axis=1).astype(BF16)
    g1 = np.asarray(inputs["g1"], dtype=F32)
    b1 = np.asarray(inputs["b1"], dtype=F32)
    g2 = np.asarray(inputs["g2"], dtype=F32)
    b2 = np.asarray(inputs["b2"], dtype=F32)
    vecs = np.zeros((128, 8), dtype=F32)
    vecs[:, 0] = np.asarray(inputs["bn_g"], dtype=F32)
    vecs[:, 1] = np.asarray(inputs["bn_b"], dtype=F32)
    vecs[:, 2] = g1[:128]; vecs[:, 3] = g1[128:]
    vecs[:, 4] = b1[:128]; vecs[:, 5] = b1[128:]
    vecs[:, 6] = np.asarray(inputs["bm3"], dtype=F32)
    vecs2 = np.zeros((128, 4), dtype=F32)
    vecs2[:, 0] = g2[:128]; vecs2[:, 1] = g2[128:]
    vecs2[:, 2] = b2[:128]; vecs2[:, 3] = b2[128:]

    shared = dict(W1=bf(inputs["W1"]), W2=bf(inputs["W2"]), W3=bf(inputs["W3"]),
                  We_aug=We_aug, Wm1=bf(inputs["Wm1"]), Wm2p=Wm2p, Wm3p=Wm3p,
                  vecs=vecs, vecs2=vecs2)
    in_maps = []
    for c in range(NCORES):
        d = cores[c]
        m = dict(shared)
        m.update(big2=d["big2"], ohne=d["ohne"], small=d["small"],
                 ohen=d["ohen"], nbT=d["nbT"])
        in_maps.append(m)

    res = bass_utils.run_bass_kernel_spmd(nc, in_maps,
                                          core_ids=list(range(NCORES)))
    out = np.concatenate(
        [res.results[c]["yout"].T[:NB] for c in range(NCORES)], axis=0)
    return out.astype(F32)
